# revision 1
# baseline (speedup 1.0000x reference)
"""Trainium2 Bass kernel for supervised contrastive loss (8-core SPMD).

Math (per reference):
    f = x / max(||x||, 1e-12)            row-normalized features  [B, D]
    s = (f f^T) / TEMP                                            [B, B]
    E = exp(s) with diag zeroed
    P_i = sum_{j != i, l_j == l_i} E_ij   (positives)
    T_i = sum_{j != i} E_ij               (positives + negatives)
    loss = mean_i [ log(T_i + EPS) - log(P_i) ]

Distribution: row-block shard. Core c owns rows m in [1024c, 1024(c+1)).
Each core computes E^T blocks [j-chunk(128) x m(1024)] with j on the
partition dim, so BOTH reductions (over j) are partition-contractions and
run on the TensorEngine:
    PS1[c', m] = sum_j Y'[j, c'] * E[j, m]     (Y' = one-hot(labels) | ones)
row 100 of PS1 = T_m, and P_m = PS1[l_m, m] (recovered with a one-hot
mask + ones-matmul). Per-core scalar partial losses are summed on host.

SPMD uniformity: every core runs the identical program; per-core variation
lives entirely in the input data. Chunk arrays are rotated per core so the
core's own (diagonal) chunks are always processed at t = 0..7.
"""

import numpy as np
import ml_dtypes

TEMPERATURE = 0.07
EPS = 1e-8
B = 8192
D = 512
NCORES = 8
M = B // NCORES          # 1024 rows per core
NCH = B // 128           # 64 j-chunks of 128
BCH = M // 128           # 8 chunks belonging to the core's own block
NCLS = 100               # label classes
YC = NCLS + 1            # one-hot columns + ones column

_CACHE = {}


def _build_bass():
    import concourse.bass as bass
    import concourse.bacc as bacc
    import concourse.tile as tile
    from concourse import mybir
    from contextlib import ExitStack

    f32 = mybir.dt.float32
    bf16 = mybir.dt.bfloat16
    AF = mybir.ActivationFunctionType
    OP = mybir.AluOpType

    nc = bacc.Bacc()

    # ---- I/O ----------------------------------------------------------
    # xt[t, p, dc*128+jj] = x[jc_t*128+jj, dc*128+p]   (chunk-major x^T)
    xt_d = nc.declare_dram_parameter("xt", [NCH, 128, D], bf16, isOutput=False)
    # xnat[t, p, d] = x[jc_t*128+p, d]                 (natural row tiles)
    xnat_d = nc.declare_dram_parameter("xnat", [NCH, 128, D], bf16, isOutput=False)
    # all label/iota constants in ONE tensor (single DMA -> single wait for
    # downstream DVE ops, which only support one sync-wait in walrus):
    #   [:, 0:6464]        iota[p, t, c'] = c' - 1
    #   [:, 6464:12928]    labbc[p, t, c'] = labels[jc_t*128+p]
    #   [:, 12928:13952]   labblk[p, m] = labels[block row m]
    #   [:, 13952:13954]   iotap[p] = p - 1 as raw f32 (two bf16 slots)
    LC = NCH * YC
    labio_d = nc.declare_dram_parameter(
        "labio", [128, 2 * LC + M + 2], bf16, isOutput=False
    )
    loss_d = nc.declare_dram_parameter("loss", [1, 1], f32, isOutput=True)

    with ExitStack() as ctx:
        tc = ctx.enter_context(tile.TileContext(nc))
        const = ctx.enter_context(tc.tile_pool(name="const", bufs=1))
        xtp = ctx.enter_context(tc.tile_pool(name="xtp", bufs=4))
        xnp = ctx.enter_context(tc.tile_pool(name="xnp", bufs=4))
        sqp = ctx.enter_context(tc.tile_pool(name="sqp", bufs=4))
        lnp = ctx.enter_context(tc.tile_pool(name="lnp", bufs=2))
        ep = ctx.enter_context(tc.tile_pool(name="ep", bufs=3))
        psum = ctx.enter_context(tc.tile_pool(name="psum", bufs=3, space="PSUM"))
        accp = ctx.enter_context(tc.tile_pool(name="accp", bufs=1, space="PSUM"))

        # ---- constants / label machinery ------------------------------
        labio = const.tile([128, 2 * LC + M + 2], bf16)
        nc.sync.dma_start(out=labio[:], in_=labio_d[:])
        iota_cl = labio[:, 0:LC].rearrange("p (t c) -> p t c", c=YC)
        labbc_sb = labio[:, LC : 2 * LC].rearrange("p (t c) -> p t c", c=YC)
        labblk_sb = labio[:, 2 * LC : 2 * LC + M]
        iota_p = labio[:, 2 * LC + M : 2 * LC + M + 2].bitcast(f32)

        # Y'[p, t, c'] = (c'-1 == labels[j]) for c' in 1..100; col 0 = ones
        # (T-sum column at c'=0 so T lands on PSUM partition 0).
        yall = const.tile([128, NCH, YC], bf16)
        nc.vector.tensor_tensor(
            out=yall[:], in0=iota_cl, in1=labbc_sb, op=OP.is_equal
        )
        nc.vector.memset(yall[:, :, 0:1], 1.0)

        # YblkT[c', m] = (labels[block m] == c'-1)
        yblkt = const.tile([128, M], bf16)
        nc.vector.tensor_scalar(
            out=yblkt[:], in0=labblk_sb, scalar1=iota_p, scalar2=None,
            op0=OP.is_equal,
        )

        ones101 = const.tile([128, 1], f32)
        nc.vector.memset(ones101[:], 1.0)
        bias_ltemp = const.tile([128, 1], f32)
        nc.vector.memset(bias_ltemp[:], float(-np.log(TEMPERATURE)))
        bias_eps = const.tile([128, 1], f32)
        nc.vector.memset(bias_eps[:], EPS)

        # ---- row norms:  nsq[j] = sum_d x[j,d]^2  (grouped by 8 chunks) ---
        nsqg = [
            const.tile([128, BCH], f32, tag=f"nsq{g}", name=f"nsq{g}")
            for g in range(8)
        ]
        scaleg = [
            const.tile([128, BCH], f32, tag=f"scl{g}", name=f"scl{g}")
            for g in range(8)
        ]
        for t in range(NCH):
            xn_t = xnp.tile([128, D], bf16)
            nc.sync.dma_start(out=xn_t[:], in_=xnat_d[t])
            sq_t = sqp.tile([128, D], bf16)
            # square + free-dim reduce on DVE, keeping ACT free for the exps
            nc.vector.tensor_mul(out=sq_t[:], in0=xn_t[:], in1=xn_t[:])
            nc.vector.tensor_reduce(
                out=nsqg[t // BCH][:, t % BCH : t % BCH + 1], in_=sq_t[:],
                axis=mybir.AxisListType.X, op=OP.add,
            )

        # scale_j = 1/(||x_j|| * TEMP) = exp(-0.5*ln(nsq) - ln(TEMP))
        for g in range(8):
            ln_g = lnp.tile([128, BCH], f32)
            nc.scalar.activation(out=ln_g[:], in_=nsqg[g][:], func=AF.Ln)
            nc.scalar.activation(
                out=scaleg[g][:], in_=ln_g[:], func=AF.Exp,
                bias=bias_ltemp[:], scale=-0.5,
            )

        # ---- normalized own-block x^T:  xnT[d, m] = x^T[d, m] / ||x_m|| ---
        x8 = const.tile([128, BCH, D], bf16)
        nc.sync.dma_start(
            out=x8[:], in_=xt_d[0:BCH].rearrange("t p f -> p t f")
        )

        # block-row norms in ROW layout: nsq_row[0, m] = sum_{p,dc} xT[.,m]^2
        # via DVE square + 4 accumulated ones-matmul partition reductions.
        ones_bf = const.tile([128, 1], bf16)
        nc.vector.memset(ones_bf[:], 1.0)
        x8sq = const.tile([128, BCH, D], bf16)
        nc.vector.tensor_tensor(
            out=x8sq[:], in0=x8[:], in1=x8[:], op=OP.mult
        )
        nsqrow_ps = psum.tile([128, M], f32, tag="sim", name="nsqrow_ps")
        for dc in range(4):
            for h in range(2):
                nc.tensor.matmul(
                    nsqrow_ps[0:1, h * 512 : (h + 1) * 512],
                    lhsT=ones_bf[:, 0:1],
                    rhs=x8sq[:, h * 4 : (h + 1) * 4, dc * 128 : (dc + 1) * 128],
                    start=(dc == 0),
                    stop=(dc == 3),
                )
        lnrow = const.tile([1, M], f32)
        nc.scalar.activation(out=lnrow[:], in_=nsqrow_ps[0:1, :], func=AF.Ln)
        # row of 1/||x_m|| on partition 0 of a zeroed tile, then broadcast to
        # all partitions with a ones-matmul (K=128, rows 1..127 are zero).
        rowpad = const.tile([128, M], f32)
        nc.vector.memset(rowpad[:], 0.0)
        nc.scalar.activation(
            out=rowpad[0:1, :], in_=lnrow[:], func=AF.Exp, bias=0.0, scale=-0.5
        )
        ones_f = const.tile([128, 128], f32)
        nc.vector.memset(ones_f[:], 1.0)
        invnbc_ps = psum.tile([128, M], f32, tag="sim", name="invnbc_ps")
        for h in range(2):
            nc.tensor.matmul(
                invnbc_ps[:, h * 512 : (h + 1) * 512],
                lhsT=ones_f[:],
                rhs=rowpad[:, h * 512 : (h + 1) * 512],
                start=True,
                stop=True,
            )
        # DVE copy PSUM->SBUF so the xnt multiplies have a single
        # cross-engine dependency (the x8 DMA).
        invnbc = const.tile([128, M], f32)
        nc.vector.tensor_copy(out=invnbc[:], in_=invnbc_ps[:])
        xnt = const.tile([128, 4, M], bf16)
        for dc in range(4):
            nc.vector.tensor_tensor(
                out=xnt[:, dc, :].rearrange("p (t j) -> p t j", j=128),
                in0=x8[:, :, dc * 128 : (dc + 1) * 128],
                in1=invnbc[:].rearrange("p (t j) -> p t j", j=128),
                op=OP.mult,
            )

        # ---- main loop over j-chunks ----------------------------------
        ps1 = accp.tile([128, M], f32)  # row 0: T; rows 1..100: class sums
        for t in range(NCH):
            if t < BCH:
                lhs = x8[:, t, :]
            else:
                lhs_t = xtp.tile([128, D], bf16)
                nc.sync.dma_start(out=lhs_t[:], in_=xt_d[t])
                lhs = lhs_t[:]
            ps = psum.tile([128, M], f32, tag="sim")
            for dc in range(4):
                for h in range(2):
                    nc.tensor.matmul(
                        ps[:, h * 512 : (h + 1) * 512],
                        lhsT=lhs[:, dc * 128 : (dc + 1) * 128],
                        rhs=xnt[:, dc, h * 512 : (h + 1) * 512],
                        start=(dc == 0),
                        stop=(dc == 3),
                    )
            e_t = ep.tile([128, M], bf16)
            g, k = t // BCH, t % BCH
            nc.scalar.activation(
                out=e_t[:], in_=ps[:], func=AF.Exp, scale=scaleg[g][:, k : k + 1]
            )
            if t < BCH:
                # zero the diagonal: kill (p, m) where m - p - 128*t == 0
                nc.gpsimd.affine_select(
                    out=e_t[:], in_=e_t[:], pattern=[[1, M]],
                    compare_op=OP.not_equal, fill=0.0,
                    base=-(t * 128), channel_multiplier=-1,
                )
            for h in range(2):
                nc.tensor.matmul(
                    ps1[0:YC, h * 512 : (h + 1) * 512],
                    lhsT=yall[:, t, :],
                    rhs=e_t[:, h * 512 : (h + 1) * 512],
                    start=(t == 0),
                    stop=(t == NCH - 1),
                )

        # ---- finalize: P via one-hot mask + partition reduce ----------
        maskd = const.tile([128, M], f32)
        nc.vector.tensor_tensor(
            out=maskd[0:YC, :], in0=ps1[0:YC, :], in1=yblkt[0:YC, :], op=OP.mult
        )
        pps = psum.tile([128, M], f32, tag="sim")
        for h in range(2):
            nc.tensor.matmul(
                pps[0:1, h * 512 : (h + 1) * 512],
                lhsT=ones101[0:YC, 0:1],
                rhs=maskd[0:YC, h * 512 : (h + 1) * 512],
                start=True,
                stop=True,
            )
        ln_t = const.tile([1, M], f32)
        nc.scalar.activation(
            out=ln_t[:], in_=ps1[0:1, :], func=AF.Ln, bias=bias_eps[0:1, :]
        )
        ln_p = const.tile([1, M], f32)
        nc.scalar.activation(out=ln_p[:], in_=pps[0:1, :], func=AF.Ln)
        diff = const.tile([1, M], f32)
        nc.vector.tensor_sub(out=diff[:], in0=ln_t[:], in1=ln_p[:])
        losss = const.tile([1, 1], f32)
        nc.vector.tensor_reduce(
            out=losss[:], in_=diff[:], axis=mybir.AxisListType.X, op=OP.add
        )
        nc.sync.dma_start(out=loss_d[:], in_=losss[:])

    # Bacc.finalize() runs the wait-splitting / ldweights / act-table /
    # extended-ISA codegen passes that walrus requires.
    nc.finalize()
    return nc


def _prep_inputs(features: np.ndarray, labels: np.ndarray):
    """Shard + lay out the full inputs for the 8 cores (host marshalling)."""
    bf16 = ml_dtypes.bfloat16
    x = np.ascontiguousarray(features, dtype=np.float32)
    x_bf = x.astype(bf16)
    # chunk-major x^T: xtc[jc, p, dc*128+jj] = x[jc*128+jj, dc*128+p]
    xtc = np.ascontiguousarray(
        x_bf.reshape(NCH, 128, 4, 128).transpose(0, 3, 2, 1)
    ).reshape(NCH, 128, D)
    xnat = x_bf.reshape(NCH, 128, D)
    lab_f = labels.astype(np.float32)
    lab_ch = lab_f.reshape(NCH, 128)
    LC = NCH * YC
    iota_cl = np.broadcast_to(
        (np.arange(YC, dtype=np.float32) - 1.0)[None, None, :], (128, NCH, YC)
    )
    iota_p = (np.arange(128, dtype=np.float32) - 1.0)[:, None]
    in_maps = []
    for c in range(NCORES):
        r = np.roll(np.arange(NCH), -BCH * c)
        labio = np.empty((128, 2 * LC + M + 2), dtype=np.float32)
        labio[:, 0:LC] = iota_cl.reshape(128, LC)
        # labbc[p, t, c'] = labels[jc_t*128 + p]
        labio[:, LC : 2 * LC] = np.repeat(lab_ch[r].T, YC, axis=1)
        labio[:, 2 * LC : 2 * LC + M] = lab_f[c * M : (c + 1) * M][None, :]
        labio_bf = labio.astype(bf16)
        # last two bf16 slots per row hold the raw f32 bits of (p - 1)
        labio_bf.view(np.uint16)[:, 2 * LC + M :] = (
            iota_p.astype("<f4").view(np.uint16).reshape(128, 2)
        )
        in_maps.append(
            {
                "xt": np.ascontiguousarray(xtc[r]),
                "xnat": np.ascontiguousarray(xnat[r]),
                "labio": labio_bf,
            }
        )
    return in_maps


def kernel(features: np.ndarray, labels: np.ndarray) -> np.ndarray:
    from concourse.bass_utils import run_bass_kernel_spmd

    if "nc" not in _CACHE:
        _CACHE["nc"] = _build_bass()
    nc = _CACHE["nc"]
    in_maps = _prep_inputs(features, labels)
    res = run_bass_kernel_spmd(nc, in_maps, list(range(NCORES)))
    total = sum(float(r["loss"][0, 0]) for r in res.results)
    return np.float32(total / B)



# revision 2
# speedup vs baseline: 3.1500x; 3.1500x over previous
"""Trainium2 Bass kernel for supervised contrastive loss (8-core SPMD).

v4: host pre-normalizes rows (f = x/max(||x||,1e-12), f32 norms — matches
the reference exactly) and ships f*K quantized to a narrow dtype (fp8 by
default, 4MB/core). The same tensor serves both matmul sides:
    ps = (K f_j)·(K f_m),  e = exp(ps / (K^2 * TEMP))
so all on-device norm machinery disappears and the exp scale is a
constant. Everything else as v3: rotated chunk order so the diagonal
block is always t=0..7 (static affine_select), labels one-hots built on
device from a 32KB aux, per-core partial losses summed on host.
"""

import numpy as np
import ml_dtypes

import jax

# Persistent XLA compilation cache: the bass2jax path jits a fresh closure
# per call, so without this every kernel() invocation re-runs the full
# client-side BIR->NEFF compile (~0.4s). With it, repeat calls hit the
# cache keyed on the (identical) HLO.
jax.config.update("jax_compilation_cache_dir", "/tmp/jax_comp_cache")
jax.config.update("jax_persistent_cache_min_compile_time_secs", 0.0)
jax.config.update("jax_persistent_cache_min_entry_size_bytes", 0)

TEMPERATURE = 0.07
EPS = 1e-8
B = 8192
D = 512
NCORES = 8
M = B // NCORES          # 1024 rows per core
NCH = B // 128           # 64 j-chunks of 128
BCH = M // 128           # 8 chunks belonging to the core's own block
NCLS = 100               # label classes
YC = NCLS + 1            # one-hot columns + ones column

XDT = "float8e3"         # "bfloat16" | "float8e4" | "float8e3"
KS = {"bfloat16": 1.0, "float8e4": 16.0, "float8e3": 32.0}

_CACHE = {}


def _build_bass():
    import concourse.bass as bass
    import concourse.bacc as bacc
    import concourse.tile as tile
    from concourse import mybir
    from contextlib import ExitStack

    f32 = mybir.dt.float32
    bf16 = mybir.dt.bfloat16
    xdt = getattr(mybir.dt, XDT)
    K = KS[XDT]
    AF = mybir.ActivationFunctionType
    OP = mybir.AluOpType

    nc = bacc.Bacc()

    # ---- I/O ----------------------------------------------------------
    # xt[i, dd, dc*128+jj] = K*f[g_i*128+jj, dc*128+dd], g_i = (i+BCH*c)%NCH
    xt_d = nc.declare_dram_parameter("xt", [NCH, 128, D], xdt, isOutput=False)
    # aux[:, i] = labels[g_i*128+p]  (rotated like xt)
    aux_d = nc.declare_dram_parameter("aux", [128, NCH], f32, isOutput=False)
    labrow_d = nc.declare_dram_parameter("labrow", [1, M], f32, isOutput=False)
    loss_d = nc.declare_dram_parameter("loss", [1, 1], f32, isOutput=True)

    with ExitStack() as ctx:
        tc = ctx.enter_context(tile.TileContext(nc))
        const = ctx.enter_context(tc.tile_pool(name="const", bufs=1))
        xtp = ctx.enter_context(tc.tile_pool(name="xtp", bufs=4))
        ep = ctx.enter_context(tc.tile_pool(name="ep", bufs=3))
        psum = ctx.enter_context(tc.tile_pool(name="psum", bufs=3, space="PSUM"))
        accp = ctx.enter_context(tc.tile_pool(name="accp", bufs=1, space="PSUM"))

        aux = const.tile([128, NCH], f32)
        nc.sync.dma_start(out=aux[:], in_=aux_d[:])
        x8 = const.tile([128, BCH, D], xdt)
        nc.sync.dma_start(out=x8[:], in_=xt_d[0:BCH].rearrange("t p f -> p t f"))
        # rhs layout: xnt[dd, dc, t*128+jj] = xt[t, dd, dc*128+jj]
        xnt = const.tile([128, 4, M], xdt)
        nc.sync.dma_start(
            out=xnt[:].rearrange("p c (t j) -> p c t j", j=128),
            in_=xt_d[0:BCH].rearrange("t p (c j) -> p c t j", c=4),
        )

        ones_f = const.tile([128, 128], f32)
        nc.vector.memset(ones_f[:], 1.0)
        bias_eps = const.tile([128, 1], f32)
        nc.vector.memset(bias_eps[:], EPS)

        # ---- label machinery (tiny inputs + iota) ----------------------
        iota_c = const.tile([128, YC], f32)
        nc.gpsimd.iota(
            iota_c[:], pattern=[[1, YC]], base=-1, channel_multiplier=0,
            allow_small_or_imprecise_dtypes=True,
        )
        iota_p = const.tile([128, 1], f32)
        nc.gpsimd.iota(
            iota_p[:], pattern=[[1, 1]], base=-1, channel_multiplier=1,
            allow_small_or_imprecise_dtypes=True,
        )

        yall = const.tile([128, NCH, YC], bf16)
        for t in range(NCH):
            nc.vector.tensor_scalar(
                out=yall[:, t, :], in0=iota_c[:], scalar1=aux[:, t : t + 1],
                scalar2=None, op0=OP.is_equal,
            )
        nc.vector.memset(yall[:, :, 0:1], 1.0)

        rowpad2 = const.tile([128, M], f32)
        nc.vector.memset(rowpad2[:], 0.0)
        nc.sync.dma_start(out=rowpad2[0:1, :], in_=labrow_d[:])
        labm_ps = psum.tile([128, M], f32, tag="sim", name="labm_ps")
        for h in range(2):
            nc.tensor.matmul(
                labm_ps[:, h * 512 : (h + 1) * 512],
                lhsT=ones_f[:],
                rhs=rowpad2[:, h * 512 : (h + 1) * 512],
                start=True,
                stop=True,
            )
        labm_bc = const.tile([128, M], f32)
        nc.vector.tensor_copy(out=labm_bc[:], in_=labm_ps[:])
        yblkt = const.tile([128, M], bf16)
        nc.vector.tensor_scalar(
            out=yblkt[:], in0=labm_bc[:], scalar1=iota_p[:], scalar2=None,
            op0=OP.is_equal,
        )

        # ---- main loop over j-chunks (rotated order) -------------------
        exp_scale = float(1.0 / (K * K * TEMPERATURE))
        ps1 = accp.tile([128, M], f32)  # row 0: T; rows 1..100: class sums
        for t in range(NCH):
            if t < BCH:
                lhs = x8[:, t, :]
            else:
                lhs_t = xtp.tile([128, D], xdt)
                nc.sync.dma_start(out=lhs_t[:], in_=xt_d[t])
                lhs = lhs_t[:]
            ps = psum.tile([128, M], f32, tag="sim")
            for dc in range(4):
                for h in range(2):
                    nc.tensor.matmul(
                        ps[:, h * 512 : (h + 1) * 512],
                        lhsT=lhs[:, dc * 128 : (dc + 1) * 128],
                        rhs=xnt[:, dc, h * 512 : (h + 1) * 512],
                        start=(dc == 0),
                        stop=(dc == 3),
                    )
            e_t = ep.tile([128, M], bf16)
            nc.scalar.activation(
                out=e_t[:], in_=ps[:], func=AF.Exp, scale=exp_scale
            )
            if t < BCH:
                nc.gpsimd.affine_select(
                    out=e_t[:], in_=e_t[:], pattern=[[1, M]],
                    compare_op=OP.not_equal, fill=0.0,
                    base=-(t * 128), channel_multiplier=-1,
                )
            for h in range(2):
                nc.tensor.matmul(
                    ps1[0:YC, h * 512 : (h + 1) * 512],
                    lhsT=yall[:, t, :],
                    rhs=e_t[:, h * 512 : (h + 1) * 512],
                    start=(t == 0),
                    stop=(t == NCH - 1),
                )

        # ---- finalize: P via one-hot mask + partition reduce -----------
        maskd = const.tile([128, M], f32)
        nc.vector.tensor_tensor(
            out=maskd[0:YC, :], in0=ps1[0:YC, :], in1=yblkt[0:YC, :], op=OP.mult
        )
        pps = psum.tile([128, M], f32, tag="sim")
        for h in range(2):
            nc.tensor.matmul(
                pps[0:1, h * 512 : (h + 1) * 512],
                lhsT=ones_f[0:YC, 0:1],
                rhs=maskd[0:YC, h * 512 : (h + 1) * 512],
                start=True,
                stop=True,
            )
        ln_t = const.tile([1, M], f32)
        nc.scalar.activation(
            out=ln_t[:], in_=ps1[0:1, :], func=AF.Ln, bias=bias_eps[0:1, :]
        )
        ln_p = const.tile([1, M], f32)
        nc.scalar.activation(out=ln_p[:], in_=pps[0:1, :], func=AF.Ln)
        diff = const.tile([1, M], f32)
        nc.vector.tensor_sub(out=diff[:], in0=ln_t[:], in1=ln_p[:])
        losss = const.tile([1, 1], f32)
        nc.vector.tensor_reduce(
            out=losss[:], in_=diff[:], axis=mybir.AxisListType.X, op=OP.add
        )
        nc.sync.dma_start(out=loss_d[:], in_=losss[:])

    nc.finalize()
    return nc


def _prep_inputs(features: np.ndarray, labels: np.ndarray):
    """Shard + lay out the full inputs for the 8 cores (host marshalling)."""
    from concourse import mybir

    np_xdt = mybir.dt.np(getattr(mybir.dt, XDT))
    K = KS[XDT]
    x_f = np.asarray(features, dtype=np.float32)
    nrm = np.maximum(np.sqrt((x_f * x_f).sum(axis=1, keepdims=True)), 1e-12)
    fq = np.clip((x_f / nrm) * np.float32(K), -15.0, 15.0).astype(np_xdt)
    # chunk-major f^T: xt[g, dd, dc*128+jj] = fq[g*128+jj, dc*128+dd]
    xt = np.ascontiguousarray(
        fq.reshape(NCH, 128, 4, 128).transpose(0, 3, 2, 1)
    ).reshape(NCH, 128, D)
    lab_f = labels.astype(np.float32)
    lab_ch = lab_f.reshape(NCH, 128)
    in_maps = []
    for c in range(NCORES):
        r = np.roll(np.arange(NCH), -BCH * c)
        in_maps.append(
            {
                "xt": np.ascontiguousarray(xt[r]),
                "aux": np.ascontiguousarray(lab_ch[r].T),
                "labrow": np.ascontiguousarray(
                    lab_f[c * M : (c + 1) * M].reshape(1, M)
                ),
            }
        )
    return in_maps


def kernel(features: np.ndarray, labels: np.ndarray) -> np.ndarray:
    from concourse.bass_utils import run_bass_kernel_spmd

    if "nc" not in _CACHE:
        _CACHE["nc"] = _build_bass()
    nc = _CACHE["nc"]
    in_maps = _prep_inputs(features, labels)
    res = run_bass_kernel_spmd(nc, in_maps, list(range(NCORES)))
    total = sum(float(r["loss"][0, 0]) for r in res.results)
    return np.float32(total / B)


# revision 3
# speedup vs baseline: 3.2569x; 1.0340x over previous
"""Supervised contrastive loss on ONE NeuronCore (wire-bound regime).

Every core needs the full feature matrix for the f f^T bilinear form, so
an 8-core SPMD layout ships 8 copies over the axon tunnel; the tunnel is
the bottleneck (exec ~1ms). One core ships ONE 4MB fp8 copy and loops
over the 8 column groups internally.
"""

import numpy as np
import ml_dtypes

import jax

jax.config.update("jax_compilation_cache_dir", "/tmp/jax_comp_cache")
jax.config.update("jax_persistent_cache_min_compile_time_secs", 0.0)
jax.config.update("jax_persistent_cache_min_entry_size_bytes", 0)

TEMPERATURE = 0.07
EPS = 1e-8
B = 8192
D = 512
NCORES = 1
M = 1024                 # columns per group
NG = B // M              # 8 column groups
NCH = B // 128           # 64 j-chunks
NCLS = 100
YC = NCLS + 1
XDT = "float8e3"
K_SCALE = 32.0

_CACHE = {}


def _build_bass():
    import concourse.bacc as bacc
    import concourse.tile as tile
    from concourse import mybir
    from contextlib import ExitStack

    f32 = mybir.dt.float32
    bf16 = mybir.dt.bfloat16
    xdt = getattr(mybir.dt, XDT)
    AF = mybir.ActivationFunctionType
    OP = mybir.AluOpType

    nc = bacc.Bacc()

    xt_d = nc.declare_dram_parameter("xt", [NCH, 128, D], xdt, isOutput=False)
    aux_d = nc.declare_dram_parameter("aux", [128, NCH], f32, isOutput=False)
    labrow_d = nc.declare_dram_parameter("labrow", [1, B], f32, isOutput=False)
    loss_d = nc.declare_dram_parameter("loss", [1, 1], f32, isOutput=True)

    with ExitStack() as ctx:
        tc = ctx.enter_context(tile.TileContext(nc))
        const = ctx.enter_context(tc.tile_pool(name="const", bufs=1))
        ep = ctx.enter_context(tc.tile_pool(name="ep", bufs=3))
        mkp = ctx.enter_context(tc.tile_pool(name="mkp", bufs=2))
        psum = ctx.enter_context(tc.tile_pool(name="psum", bufs=3, space="PSUM"))
        accp = ctx.enter_context(tc.tile_pool(name="accp", bufs=1, space="PSUM"))

        aux = const.tile([128, NCH], f32)
        nc.sync.dma_start(out=aux[:], in_=aux_d[:])
        xsb = const.tile([128, NCH, D], xdt)
        nc.sync.dma_start(out=xsb[:], in_=xt_d[:].rearrange("t p f -> p t f"))
        xnt = const.tile([128, 4, B], xdt)
        nc.sync.dma_start(
            out=xnt[:].rearrange("p c (t j) -> p c t j", j=128),
            in_=xt_d[:].rearrange("t p (c j) -> p c t j", c=4),
        )
        labrow = const.tile([1, B], f32)
        nc.sync.dma_start(out=labrow[:], in_=labrow_d[:])

        ones_f = const.tile([128, 128], f32)
        nc.vector.memset(ones_f[:], 1.0)
        ones_row = const.tile([1, 128], f32)
        nc.vector.memset(ones_row[:], 1.0)
        bias_eps = const.tile([128, 1], f32)
        nc.vector.memset(bias_eps[:], EPS)

        iota_c = const.tile([128, YC], f32)
        nc.gpsimd.iota(
            iota_c[:], pattern=[[1, YC]], base=-1, channel_multiplier=0,
            allow_small_or_imprecise_dtypes=True,
        )
        iota_p = const.tile([128, 1], f32)
        nc.gpsimd.iota(
            iota_p[:], pattern=[[1, 1]], base=-1, channel_multiplier=1,
            allow_small_or_imprecise_dtypes=True,
        )

        yall = const.tile([128, NCH, YC], bf16)
        for t in range(NCH):
            nc.vector.tensor_scalar(
                out=yall[:, t, :], in0=iota_c[:], scalar1=aux[:, t : t + 1],
                scalar2=None, op0=OP.is_equal,
            )
        nc.vector.memset(yall[:, :, 0:1], 1.0)

        # yblkt[c', m] = (labels[m] == c'-1), built per group via a K=1
        # outer-product broadcast of the label row
        yblkt = const.tile([128, B], bf16)
        for g in range(NG):
            lb_ps = psum.tile([128, M], f32, tag="sim")
            for h in range(2):
                nc.tensor.matmul(
                    lb_ps[:, h * 512 : (h + 1) * 512],
                    lhsT=ones_row[0:1, :],
                    rhs=labrow[0:1, g * M + h * 512 : g * M + (h + 1) * 512],
                    start=True,
                    stop=True,
                )
            nc.vector.tensor_scalar(
                out=yblkt[:, g * M : (g + 1) * M], in0=lb_ps[:],
                scalar1=iota_p[:], scalar2=None, op0=OP.is_equal,
            )

        exp_scale = float(1.0 / (K_SCALE * K_SCALE * TEMPERATURE))
        rowp = ctx.enter_context(tc.tile_pool(name="rowp", bufs=2))
        lossparts = const.tile([1, NG], f32)

        for g in range(NG):
            ps1 = accp.tile([128, M], f32)
            for t in range(NCH):
                ps = psum.tile([128, M], f32, tag="sim")
                for dc in range(4):
                    for h in range(2):
                        nc.tensor.matmul(
                            ps[:, h * 512 : (h + 1) * 512],
                            lhsT=xsb[:, t, dc * 128 : (dc + 1) * 128],
                            rhs=xnt[:, dc, g * M + h * 512 : g * M + (h + 1) * 512],
                            start=(dc == 0),
                            stop=(dc == 3),
                        )
                e_t = ep.tile([128, M], bf16)
                nc.scalar.activation(
                    out=e_t[:], in_=ps[:], func=AF.Exp, scale=exp_scale
                )
                if t // 8 == g:
                    nc.gpsimd.affine_select(
                        out=e_t[:], in_=e_t[:], pattern=[[1, M]],
                        compare_op=OP.not_equal, fill=0.0,
                        base=-((t % 8) * 128), channel_multiplier=-1,
                    )
                for h in range(2):
                    nc.tensor.matmul(
                        ps1[0:YC, h * 512 : (h + 1) * 512],
                        lhsT=yall[:, t, :],
                        rhs=e_t[:, h * 512 : (h + 1) * 512],
                        start=(t == 0),
                        stop=(t == NCH - 1),
                    )
            maskd = mkp.tile([128, M], f32)
            nc.vector.tensor_tensor(
                out=maskd[0:YC, :], in0=ps1[0:YC, :],
                in1=yblkt[0:YC, g * M : (g + 1) * M], op=OP.mult,
            )
            pps = psum.tile([128, M], f32, tag="sim")
            for h in range(2):
                nc.tensor.matmul(
                    pps[0:1, h * 512 : (h + 1) * 512],
                    lhsT=ones_f[0:YC, 0:1],
                    rhs=maskd[0:YC, h * 512 : (h + 1) * 512],
                    start=True,
                    stop=True,
                )
            ln_t = rowp.tile([1, M], f32, tag="lnt")
            nc.scalar.activation(
                out=ln_t[:], in_=ps1[0:1, :], func=AF.Ln, bias=bias_eps[0:1, :]
            )
            ln_p = rowp.tile([1, M], f32, tag="lnp")
            nc.scalar.activation(out=ln_p[:], in_=pps[0:1, :], func=AF.Ln)
            diff = rowp.tile([1, M], f32, tag="diff")
            nc.vector.tensor_sub(out=diff[:], in0=ln_t[:], in1=ln_p[:])
            nc.vector.tensor_reduce(
                out=lossparts[:, g : g + 1], in_=diff[:],
                axis=mybir.AxisListType.X, op=OP.add,
            )

        losss = const.tile([1, 1], f32)
        nc.vector.tensor_reduce(
            out=losss[:], in_=lossparts[:], axis=mybir.AxisListType.X, op=OP.add
        )
        nc.sync.dma_start(out=loss_d[:], in_=losss[:])

    nc.finalize()
    return nc


def _prep_inputs(features: np.ndarray, labels: np.ndarray):
    from concourse import mybir

    np_xdt = mybir.dt.np(getattr(mybir.dt, XDT))
    x_f = np.asarray(features, dtype=np.float32)
    nrm = np.maximum(np.sqrt((x_f * x_f).sum(axis=1, keepdims=True)), 1e-12)
    fq = np.clip((x_f / nrm) * np.float32(K_SCALE), -15.0, 15.0).astype(np_xdt)
    xt = np.ascontiguousarray(
        fq.reshape(NCH, 128, 4, 128).transpose(0, 3, 2, 1)
    ).reshape(NCH, 128, D)
    lab_f = labels.astype(np.float32)
    return [
        {
            "xt": xt,
            "aux": np.ascontiguousarray(lab_f.reshape(NCH, 128).T),
            "labrow": np.ascontiguousarray(lab_f.reshape(1, B)),
        }
    ]


def kernel(features: np.ndarray, labels: np.ndarray) -> np.ndarray:
    from concourse.bass_utils import run_bass_kernel_spmd

    if "nc" not in _CACHE:
        _CACHE["nc"] = _build_bass()
    nc = _CACHE["nc"]
    in_maps = _prep_inputs(features, labels)
    res = run_bass_kernel_spmd(nc, in_maps, [0])
    return np.float32(float(res.results[0]["loss"][0, 0]) / B)
